# revision 6
# baseline (speedup 1.0000x reference)
"""Trainium2 Bass kernel for batched cross-attention.

Problem (hardcoded shapes):
  img_embeds:          (8, 4096, 512)  f32
  text_embeds:         (8, 512, 768)   f32
  text_attention_mask: (8, 512)        i32
  Wq (512,512), Wk (512,768), Wv (512,768), Wo (512,512), bo (512,)
  out:                 (8, 4096, 512)  f32

Sharding: data-parallel over batch B=8 -> one batch element per NeuronCore
(8 cores). Weights replicated. No collectives needed.

Host<->device traffic is the wall-clock bottleneck, so the host path is
organized around minimizing per-call bytes and copies:
  - img is shipped as fp16 (half the bytes; quantization error ~4e-4 vs the
    2e-2 tolerance), reshaped as a zero-copy view for sharding.
  - text is compacted to the valid tokens (<=384 of 512) and shipped fp16.
  - weights are shipped once, cached device-resident across calls, guarded
    by an exact memcmp against the previous call's values.
  - the output buffers the Bass program writes are allocated on device
    inside the jitted function (no 64MB host zeros upload).
  - the output comes back fp16 (32MB instead of 64MB) and is upcast to f32
    on the host.

Per-core algorithm (all layouts chosen so the softmax denominator comes for
free and no transposes of big intermediates are needed):
  - transpose t (512x768) and the weights once; K^T = Wk^T-matmuls,
    V = t-matmuls (V stored per-head with an appended ones-column).
  - per 512-query block: PE-transpose x chunk (fp16 -> fp16 PSUM view),
    Q^T = Wq^T @ x^T.
  - per head: scores^T[j,i] = K_h^T.T @ Q_h^T (K=64), then
    exp(scale*s) on ACT, then attended^T[vd,i] = V_ext.T @ exp accumulated
    over j chunks (mask folded multiplicatively into V_ext rows).
    Row 64 of attended^T is the softmax denominator (ones column of V_ext).
    reciprocal on DVE, broadcast-normalize on DVE.
  - Y[i, od] = attn^T.T @ Wo^T (+ bo via a K=1 accumulation matmul),
    downcast to fp16 on ACT, DMA out.

Matmuls run as float32r (full fp32 data; 1 cycle/row on TRN2 when the
moving free dim >= 256). fp16 is only a transport format: everything is
upcast to f32 on the PE-transpose eviction path.
"""

import ctypes
from contextlib import ExitStack

import numpy as np

import concourse.bass as bass
import concourse.tile as tile
from concourse import bacc, mybir
from concourse.masks import make_identity

F32 = mybir.dt.float32
F32R = mybir.dt.float32r
F16 = mybir.dt.float16

B, N_IMG, N_TXT = 8, 4096, 512
IMG_DIM, TEXT_DIM, H, HD = 512, 768, 8, 64
SCALE = float((TEXT_DIM // H) ** -0.5)
P = 128
N_CORES = 8

IB = N_IMG // 512  # 8 query blocks of 512
NJ = 384  # compacted key count (3 chunks of 128); falls back to 512 if exceeded


def _r(ap):
    """fp32 -> float32r view for full-rate PE matmuls."""
    return ap.bitcast(F32R)


def _build_nc(nj: int = NJ, tiny: bool = False, repeat: int = 1) -> bass.Bass:
    njc = nj // P
    nc = bacc.Bacc("TRN2", target_bir_lowering=False, debug=False)

    img = nc.dram_tensor("img", [N_IMG, IMG_DIM], F16, kind="ExternalInput").ap()
    txt = nc.dram_tensor("txt", [nj, TEXT_DIM], F16, kind="ExternalInput").ap()
    msk = nc.dram_tensor("msk", [nj], F32, kind="ExternalInput").ap()
    wq = nc.dram_tensor("wq", [IMG_DIM, IMG_DIM], F16, kind="ExternalInput").ap()
    wk = nc.dram_tensor("wk", [IMG_DIM, TEXT_DIM], F16, kind="ExternalInput").ap()
    wv = nc.dram_tensor("wv", [IMG_DIM, TEXT_DIM], F16, kind="ExternalInput").ap()
    wo = nc.dram_tensor("wo", [IMG_DIM, IMG_DIM], F16, kind="ExternalInput").ap()
    bo = nc.dram_tensor("bo", [IMG_DIM], F32, kind="ExternalInput").ap()
    out = nc.dram_tensor("out", [N_IMG, IMG_DIM], F16, kind="ExternalOutput").ap()

    with tile.TileContext(nc) as tc:
        with ExitStack() as ctx:
            if tiny:
                with tc.tile_pool(name="tp", bufs=1) as tp:
                    tt = tp.tile([P, 512], F16, tag="tt")
                    nc.sync.dma_start(tt, img[:P, :])
                    nc.sync.dma_start(out[:P, :], tt)
            else:
                _body(ctx, tc, img, txt, msk, wq, wk, wv, wo, bo, out, njc, repeat)
    nc.compile()
    return nc


def _body(ctx, tc, img, txt, msk, wq, wk, wv, wo, bo, out, njc, repeat=1):
    nc = tc.nc
    Exp = mybir.ActivationFunctionType.Exp

    img_r = img.rearrange("(n p) d -> p n d", p=P)  # n = 32 row-chunks
    out_r = out.rearrange("(n p) d -> p n d", p=P)

    const = ctx.enter_context(tc.tile_pool(name="const", bufs=1))
    ps = ctx.enter_context(tc.tile_pool(name="ps", bufs=8, space="PSUM"))

    identity = const.tile([P, P], F32, tag="identity")
    make_identity(nc, identity)
    identity_h = const.tile([P, P], F16, tag="identity_h")
    make_identity(nc, identity_h)

    # ---- constants / weights (transposed into [contract-dim, free] layouts)
    WqT = const.tile([P, 4, 512], F32R, tag="WqT")  # [d, qd]
    WoT = const.tile([P, 4, 512], F32R, tag="WoT")  # [c, od]
    WkT = const.tile([P, 6, 512], F32R, tag="WkT")  # [td, kd]
    WvT = const.tile([P, 6, 512], F32R, tag="WvT")  # [td, vd]
    tT = const.tile([P, 6, njc * P], F32R, tag="tT")    # [td, j]
    KT = const.tile([P, 4, njc * P], F32R, tag="KT")    # [kd, j]
    Vx = const.tile([P, njc, H, 2 * HD], F32R, tag="Vx")  # [j%, jc, h, vd|mask]
    bo_sb = const.tile([1, 512], F32, tag="bo_sb")
    bo_r = const.tile([1, 512], F32R, tag="bo_r")
    ones = const.tile([1, P], F32R, tag="ones")
    ones_f = const.tile([P, HD], F32, tag="ones_f")
    nc.any.memset(ones_f, 1.0)
    # mask as multiplicative factor on V_ext rows, laid out [p, jc].
    # Load contiguously as [njc, 128] (njc descriptors) and PE-transpose;
    # a direct [p, jc] DMA would be njc*128 4-byte descriptors.
    maskb_f = const.tile([P, njc], F32, tag="mf")
    mask_row = const.tile([njc, P], F32, tag="mrow")
    nc.sync.dma_start(mask_row, msk.rearrange("(c p) -> c p", p=P))
    mps = ps.tile([P, njc], F32, tag="ps", bufs=6, name="mps")
    nc.tensor.transpose(mps, mask_row, identity[:njc, :njc])
    nc.vector.tensor_copy(maskb_f, mps)
    nc.vector.tensor_copy(ones, ones_f[0:1, 0:1].broadcast_to([1, P]))
    for jc in range(njc):
        nc.vector.tensor_scalar_mul(
            Vx[:, jc, :, HD:],
            ones_f[:, None, :].broadcast_to([P, H, HD]),
            maskb_f[:, jc : jc + 1],
        )
    nc.gpsimd.dma_start(bo_sb, bo.unsqueeze(0))
    nc.vector.tensor_copy(bo_r, bo_sb)

    def transpose_in(dst, src_chunks, n_out_chunks, n_in_chunks, evict_engine):
        """dst[p, oc, ic*128+q] = src[q, ic, oc*128+p]; src is fp16,
        transposed through PE into an fp16 view of an f32 PSUM tile, then
        upcast to f32 on eviction."""
        for oc in range(n_out_chunks):
            pst = ps.tile([P, 512], F32, tag="ps", bufs=6, name=f"pst_{oc}")
            pst16 = pst.bitcast(F16)
            for ic in range(n_in_chunks):
                nc.tensor.transpose(
                    pst16[:, ic * P : (ic + 1) * P],
                    src_chunks[:, ic, oc * P : (oc + 1) * P],
                    identity_h,
                )
            evict_engine.tensor_copy(
                dst[:, oc, : n_in_chunks * P], pst16[:, : n_in_chunks * P]
            )

    # ---- one-time setup: text/weight transposes, K^T, V
    wload = ctx.enter_context(tc.tile_pool(name="wload", bufs=2))
    t_sb = wload.tile([P, 4, 768], F16, tag="wl")
    nc.sync.dma_start(t_sb[:, :njc, :], txt.rearrange("(c p) d -> p c d", p=P))
    transpose_in(tT, t_sb[:, :njc, :], 6, njc, nc.vector)

    wk_sb = wload.tile([P, 4, 768], F16, tag="wl")
    nc.scalar.dma_start(wk_sb, wk.rearrange("(c p) d -> p c d", p=P))
    transpose_in(WkT, wk_sb, 6, 4, nc.vector)

    wq_sb = wload.tile([P, 4, 768], F16, tag="wl")
    nc.sync.dma_start(wq_sb[:, :, :512], wq.rearrange("(c p) d -> p c d", p=P))
    transpose_in(WqT, wq_sb[:, :, :512], 4, 4, nc.vector)

    wv_sb = wload.tile([P, 4, 768], F16, tag="wl")
    nc.scalar.dma_start(wv_sb, wv.rearrange("(c p) d -> p c d", p=P))
    transpose_in(WvT, wv_sb, 6, 4, nc.vector)

    wo_sb = wload.tile([P, 4, 768], F16, tag="wl")
    nc.sync.dma_start(wo_sb[:, :, :512], wo.rearrange("(c p) d -> p c d", p=P))
    transpose_in(WoT, wo_sb[:, :, :512], 4, 4, nc.vector)

    # K^T[kd, j] = sum_td WkT[td, kd] * tT[td, j]
    for kc in range(4):
        pkt = ps.tile([P, 512], F32, tag="ps", bufs=6, name=f"pkt_{kc}")
        for t6 in range(6):
            nc.tensor.matmul(
                pkt[:, : njc * P],
                WkT[:, t6, kc * P : (kc + 1) * P],
                tT[:, t6, :],
                start=(t6 == 0),
                stop=(t6 == 5),
            )
        nc.vector.tensor_copy(KT[:, kc, :], pkt[:, : njc * P])

    # V[j, vd] = sum_td tT[td, j] * WvT[td, vd]; per-head columns, mask applied
    for jc in range(njc):
        pv = ps.tile([P, 512], F32, tag="ps", bufs=6, name=f"pv_{jc}")
        for t6 in range(6):
            nc.tensor.matmul(
                pv,
                tT[:, t6, jc * P : (jc + 1) * P],
                WvT[:, t6, :],
                start=(t6 == 0),
                stop=(t6 == 5),
            )
        nc.vector.tensor_scalar_mul(
            Vx[:, jc, :, :HD],
            pv.rearrange("p (h v) -> p h v", h=H),
            maskb_f[:, jc : jc + 1],
        )

    # ---- pipelined pools for the main loop
    xload = ctx.enter_context(tc.tile_pool(name="xload", bufs=2))
    xtp = ctx.enter_context(tc.tile_pool(name="xtp", bufs=2))
    qtp = ctx.enter_context(tc.tile_pool(name="qtp", bufs=2))
    exp = ctx.enter_context(tc.tile_pool(name="exw", bufs=3))
    anp = ctx.enter_context(tc.tile_pool(name="anp", bufs=2))
    ysp = ctx.enter_context(tc.tile_pool(name="ysp", bufs=3))
    rcp = ctx.enter_context(tc.tile_pool(name="rcp", bufs=3))

    def _main_loop():
      for ib in range(IB):
        x_sb = xload.tile([P, 4, 512], F16, tag="x")
        nc.sync.dma_start(x_sb, img_r[:, ib * 4 : (ib + 1) * 4, :])

        # x^T for this 512-query block
        xT = xtp.tile([P, 4, 512], F32R, tag="xT")  # [d, i]
        transpose_in(xT, x_sb, 4, 4, nc.vector)

        # Q^T[qd, i] = sum_d WqT[d, qd] * xT[d, i]
        qt = qtp.tile([P, 4, 512], F32R, tag="qt")  # [qd, i]
        for qc in range(4):
            pq = ps.tile([P, 512], F32, tag="ps", bufs=6, name=f"pq_{qc}")
            for dc in range(4):
                nc.tensor.matmul(
                    pq,
                    WqT[:, dc, qc * P : (qc + 1) * P],
                    xT[:, dc, :],
                    start=(dc == 0),
                    stop=(dc == 3),
                )
            nc.vector.tensor_copy(qt[:, qc, :], pq)

        attn = anp.tile([P, 4, 512], F32R, tag="attn")  # [c, i] normalized att^T

        def head_scores(h):
            po = (h % 2) * HD
            hc = h // 2
            qh = qt[po : po + HD, hc, :]  # [64, 512]
            ex = exp.tile([P, njc, 512], F32R, tag="ex", name="ex")
            for jc in range(njc):
                sc = ps.tile([P, 512], F32, tag="ps", bufs=6, name=f"sc_{jc}")
                nc.tensor.matmul(
                    sc,
                    KT[po : po + HD, hc, jc * P : (jc + 1) * P],
                    qh,
                )
                nc.scalar.activation(ex[:, jc, :], sc, Exp, scale=SCALE)
            return ex

        def head_attend(h, ex):
            po = (h % 2) * HD
            hc = h // 2
            at = ps.tile([P, 512], F32, tag="at", bufs=2, name="at")
            for jc in range(njc):
                nc.tensor.matmul(
                    at,
                    Vx[:, jc, h, :],
                    ex[:, jc, :],
                    start=(jc == 0),
                    stop=(jc == njc - 1),
                )
            # rows [HD:2*HD] of `at` are the softmax denominator, replicated
            rec = rcp.tile([HD, 512], F32, tag="rec")
            nc.vector.reciprocal(rec, at[HD:, :])
            nc.vector.tensor_mul(attn[po : po + HD, hc, :], at[:HD, :], rec)

        # software pipeline: head h's scores/exp overlap head h-1's attend
        prev = None
        for h in range(H):
            ex = head_scores(h)
            if prev is not None:
                head_attend(prev[0], prev[1])
            prev = (h, ex)
        head_attend(prev[0], prev[1])

        # Y[i, od] = sum_c attn[c, i] * WoT[c, od] + bo
        for mc in range(4):
            py = ps.tile([P, 512], F32, tag="ps", bufs=6, name=f"py_{mc}")
            for cc in range(4):
                nc.tensor.matmul(
                    py,
                    attn[:, cc, mc * P : (mc + 1) * P],
                    WoT[:, cc, :],
                    start=(cc == 0),
                    stop=False,
                )
            nc.tensor.matmul(py, ones[0:1, :], bo_r, start=False, stop=True)
            y_sb = ysp.tile([P, 512], F16, tag="y")
            nc.scalar.copy(y_sb, py)
            nc.scalar.dma_start(out_r[:, ib * 4 + mc, :], y_sb)

    if repeat == 1:
        _main_loop()
    else:
        with tc.For_i(0, repeat, 1):
            _main_loop()


# ---------------------------------------------------------------------------
# host-side runner


_libc = ctypes.CDLL("libc.so.6", use_errno=True)
_libc.memcmp.argtypes = [ctypes.c_void_p, ctypes.c_void_p, ctypes.c_size_t]
_libc.memcmp.restype = ctypes.c_int


def _memeq(a: np.ndarray, b: np.ndarray) -> bool:
    if a.shape != b.shape or a.dtype != b.dtype:
        return False
    return (
        _libc.memcmp(
            ctypes.c_void_p(a.ctypes.data),
            ctypes.c_void_p(b.ctypes.data),
            a.nbytes,
        )
        == 0
    )


_MESH = None


def _mesh():
    global _MESH
    if _MESH is None:
        import jax
        from jax.sharding import Mesh

        _MESH = Mesh(np.asarray(jax.devices()[:N_CORES]), ("core",))
    return _MESH


_RUNNERS = {}


def _get_runner(nj: int = NJ, repeat: int = 1):
    """Build the Bass program once per (nj, repeat) and wrap it in a cached
    8-core shard_map jit. Output buffers are allocated on device inside the
    jitted body (no host zeros upload)."""
    key = (nj, repeat)
    if key in _RUNNERS:
        return _RUNNERS[key]

    import jax
    import jax.numpy as jnp
    from jax.sharding import PartitionSpec
    from jax.experimental.shard_map import shard_map
    from concourse import bass2jax

    nc = _build_nc(nj if nj > 0 else NJ, tiny=(nj <= 0), repeat=repeat)
    bass2jax.install_neuronx_cc_hook()

    partition_name = nc.partition_id_tensor.name if nc.partition_id_tensor else None
    in_names = []
    out_names = []
    out_avals = []
    out_shapes = []
    for alloc in nc.m.functions[0].allocations:
        if not isinstance(alloc, mybir.MemoryLocationSet):
            continue
        name = alloc.memorylocations[0].name
        if alloc.kind == "ExternalInput":
            if name != partition_name:
                in_names.append(name)
        elif alloc.kind == "ExternalOutput":
            shape = tuple(alloc.tensor_shape)
            dtype = mybir.dt.np(alloc.dtype)
            out_names.append(name)
            out_avals.append(jax.core.ShapedArray(shape, dtype))
            out_shapes.append((shape, dtype))
    n_params = len(in_names)
    n_outs = len(out_names)
    all_names = list(in_names) + list(out_names)
    if partition_name is not None:
        all_names.append(partition_name)

    def _bodyfn(*args):
        operands = list(args)
        if partition_name is not None:
            operands.append(bass2jax.partition_id_tensor())
        outs = bass2jax._bass_exec_p.bind(
            *operands,
            out_avals=tuple(out_avals),
            in_names=tuple(all_names),
            out_names=tuple(out_names),
            lowering_input_output_aliases=(),
            sim_require_finite=True,
            sim_require_nnan=True,
            nc=nc,
        )
        return tuple(outs)

    sharded = jax.jit(
        shard_map(
            _bodyfn,
            mesh=_mesh(),
            in_specs=(PartitionSpec("core"),) * (n_params + n_outs),
            out_specs=(PartitionSpec("core"),) * n_outs,
            check_rep=False,
        ),
        keep_unused=True,
    )

    _RUNNERS[key] = (sharded, in_names, out_names, out_shapes, nc)
    return _RUNNERS[key]


_ZCACHE = {}


def _zeros_dev(out_shapes):
    """Device-resident, non-donated ballast for the Bass program's output
    operands. The neuron lowering never reads their content (the NEFF binds
    its outputs to the result buffers), so one cached array serves every
    call with zero per-call transfer."""
    key = tuple(out_shapes)
    if key in _ZCACHE:
        return _ZCACHE[key]
    import jax
    import jax.numpy as jnp
    from jax.sharding import NamedSharding, PartitionSpec

    sh = NamedSharding(_mesh(), PartitionSpec("core"))
    zs = []
    for shape, dtype in out_shapes:
        gshape = (N_CORES * shape[0],) + tuple(shape[1:])
        z = jax.jit(lambda g=gshape, d=dtype: jnp.zeros(g, d), out_shardings=sh)()
        zs.append(z)
    jax.block_until_ready(zs)
    _ZCACHE[key] = zs
    return zs


def _compact_text(text_embeds: np.ndarray, msk: np.ndarray):
    """Gather valid text tokens (mask != 0) to the front, pad to NJ, fp16.
    Softmax gives masked tokens zero weight, so dropping them is exact.
    Falls back to the uncompacted 512-key layout if some batch has > NJ
    valid tokens. Returns (nj, txt16 (B*nj,768), val (B*nj,) f32)."""
    t = np.asarray(text_embeds)
    valid = msk != 0
    if valid.sum(axis=1).max() <= NJ:
        nj = NJ
        txt16 = np.zeros((B, NJ, TEXT_DIM), np.float16)
        val = np.zeros((B, NJ), np.float32)
        for b in range(B):
            ix = np.nonzero(valid[b])[0]
            txt16[b, : len(ix)] = t[b][ix]
            val[b, : len(ix)] = 1.0
    else:
        nj = N_TXT
        txt16 = np.asarray(t, dtype=np.float16)
        val = valid.astype(np.float32)
    return nj, txt16.reshape(B * nj, TEXT_DIM), val.reshape(B * nj)


_WCACHE = {}


def _weights_dev(Wq, Wk, Wv, Wo, bo):
    """fp16 weights, replicated per core, cached device-resident. An exact
    memcmp against the previous call's f32 values guards the cache."""
    import jax
    from jax.sharding import NamedSharding, PartitionSpec

    sh = NamedSharding(_mesh(), PartitionSpec("core"))
    out = {}
    for name, w, dt in (
        ("wq", Wq, np.float16),
        ("wk", Wk, np.float16),
        ("wv", Wv, np.float16),
        ("wo", Wo, np.float16),
        ("bo", bo, np.float32),
    ):
        w = np.ascontiguousarray(np.asarray(w, dtype=np.float32))
        ent = _WCACHE.get(name)
        if ent is not None and _memeq(ent[0], w):
            out[name] = ent[1]
            continue
        wc = np.asarray(w, dtype=dt)
        rep = np.broadcast_to(wc, (N_CORES,) + wc.shape).reshape(
            (N_CORES * wc.shape[0],) + wc.shape[1:]
        )
        dev = jax.device_put(rep, sh)
        jax.block_until_ready(dev)
        _WCACHE[name] = (w.copy(), dev)
        out[name] = dev
    return out


def _prep_inputs(img_embeds, text_embeds, text_attention_mask, Wq, Wk, Wv, Wo, bo):
    """Returns (feed dict name->global array, nj)."""
    img = np.ascontiguousarray(np.asarray(img_embeds, dtype=np.float32))
    msk = np.asarray(text_attention_mask)
    nj, txt16, val = _compact_text(text_embeds, msk)
    img16 = img.reshape(B * N_IMG, IMG_DIM).astype(np.float16)
    feed = {"img": img16, "txt": txt16, "msk": val}
    feed.update(_weights_dev(Wq, Wk, Wv, Wo, bo))
    return feed, nj


def kernel(img_embeds, text_embeds, text_attention_mask, Wq, Wk, Wv, Wo, bo):
    feed, nj = _prep_inputs(
        img_embeds, text_embeds, text_attention_mask, Wq, Wk, Wv, Wo, bo
    )
    sharded, in_names, out_names, out_shapes, _ = _get_runner(nj)
    outs = sharded(*(feed[n] for n in in_names), *_zeros_dev(out_shapes))
    out16 = np.asarray(outs[out_names.index("out")])
    return out16.astype(np.float32).reshape(B, N_IMG, IMG_DIM)


def bench_repeat(feed, nj: int = NJ, repeat: int = 25, iters: int = 12):
    """Device-time via an in-NEFF For_i repeat loop: (t[repeat] - t[1]) /
    (repeat - 1). The repeat variant runs the whole main loop `repeat` times
    on device inside one dispatch, so the delta is pure device time."""
    import time
    import jax
    from jax.sharding import NamedSharding, PartitionSpec

    sh = NamedSharding(_mesh(), PartitionSpec("core"))
    runs = {}
    for rep in (1, repeat):
        sharded, in_names, _, out_shapes, _ = _get_runner(nj, rep)
        dev_in = [
            a if isinstance(a, jax.Array) else jax.device_put(a, sh)
            for a in (feed[n] for n in in_names)
        ] + list(_zeros_dev(out_shapes))
        jax.block_until_ready(dev_in)
        o = sharded(*dev_in)
        jax.block_until_ready(o)
        runs[rep] = (sharded, dev_in)

    times = {1: [], repeat: []}
    for _ in range(iters):
        for rep in (1, repeat):
            sharded, dev_in = runs[rep]
            t0 = time.perf_counter()
            o = sharded(*dev_in)
            jax.block_until_ready(o)
            times[rep].append(time.perf_counter() - t0)
    per = (min(times[repeat]) - min(times[1])) / (repeat - 1)
    return per, times


# revision 8
# speedup vs baseline: 5.6716x; 5.6716x over previous
"""Trainium2 Bass kernel for batched cross-attention.

Problem (hardcoded shapes):
  img_embeds:          (8, 4096, 512)  f32
  text_embeds:         (8, 512, 768)   f32
  text_attention_mask: (8, 512)        i32
  Wq (512,512), Wk (512,768), Wv (512,768), Wo (512,512), bo (512,)
  out:                 (8, 4096, 512)  f32

Sharding: data-parallel over batch B=8 -> one batch element per NeuronCore
(8 cores). Weights replicated. No collectives needed.

Host<->device traffic is the wall-clock bottleneck, so the host path is
organized around minimizing per-call bytes and copies:
  - img is shipped as fp16 (half the bytes; quantization error ~4e-4 vs the
    2e-2 tolerance), reshaped as a zero-copy view for sharding.
  - text is compacted to the valid tokens (<=384 of 512) and shipped fp16.
  - weights are shipped once, cached device-resident across calls, guarded
    by an exact memcmp against the previous call's values.
  - the output buffers the Bass program writes are allocated on device
    inside the jitted function (no 64MB host zeros upload).
  - the output comes back fp16 (32MB instead of 64MB) and is upcast to f32
    on the host.

Per-core algorithm (all layouts chosen so the softmax denominator comes for
free and no transposes of big intermediates are needed):
  - transpose t (512x768) and the weights once; K^T = Wk^T-matmuls,
    V = t-matmuls (V stored per-head with an appended ones-column).
  - per 512-query block: PE-transpose x chunk (fp16 -> fp16 PSUM view),
    Q^T = Wq^T @ x^T.
  - per head: scores^T[j,i] = K_h^T.T @ Q_h^T (K=64), then
    exp(scale*s) on ACT, then attended^T[vd,i] = V_ext.T @ exp accumulated
    over j chunks (mask folded multiplicatively into V_ext rows).
    Row 64 of attended^T is the softmax denominator (ones column of V_ext).
    reciprocal on DVE, broadcast-normalize on DVE.
  - Y[i, od] = attn^T.T @ Wo^T (+ bo via a K=1 accumulation matmul),
    downcast to fp16 on ACT, DMA out.

Matmuls run as float32r (full fp32 data; 1 cycle/row on TRN2 when the
moving free dim >= 256). fp16 is only a transport format: everything is
upcast to f32 on the PE-transpose eviction path.
"""

import ctypes
from contextlib import ExitStack

import numpy as np

import concourse.bass as bass
import concourse.tile as tile
from concourse import bacc, mybir
from concourse.masks import make_identity

F32 = mybir.dt.float32
F32R = mybir.dt.float32r
F16 = mybir.dt.float16

B, N_IMG, N_TXT = 8, 4096, 512
IMG_DIM, TEXT_DIM, H, HD = 512, 768, 8, 64
SCALE = float((TEXT_DIM // H) ** -0.5)
P = 128
N_CORES = 8

IB = N_IMG // 512  # 8 query blocks of 512
NJ = 384  # compacted key count (3 chunks of 128); falls back to 512 if exceeded


def _r(ap):
    """fp32 -> float32r view for full-rate PE matmuls."""
    return ap.bitcast(F32R)


def _build_nc(nj: int = NJ, tiny: bool = False, repeat: int = 1,
              xpose16: bool = True, out16: bool = True) -> bass.Bass:
    njc = nj // P
    nc = bacc.Bacc("TRN2", target_bir_lowering=False, debug=False)

    img = nc.dram_tensor("img", [N_IMG, IMG_DIM], F16, kind="ExternalInput").ap()
    txt = nc.dram_tensor("txt", [nj, TEXT_DIM], F16, kind="ExternalInput").ap()
    msk = nc.dram_tensor("msk", [nj], F32, kind="ExternalInput").ap()
    wq = nc.dram_tensor("wq", [IMG_DIM, IMG_DIM], F16, kind="ExternalInput").ap()
    wk = nc.dram_tensor("wk", [IMG_DIM, TEXT_DIM], F16, kind="ExternalInput").ap()
    wv = nc.dram_tensor("wv", [IMG_DIM, TEXT_DIM], F16, kind="ExternalInput").ap()
    wo = nc.dram_tensor("wo", [IMG_DIM, IMG_DIM], F16, kind="ExternalInput").ap()
    bo = nc.dram_tensor("bo", [IMG_DIM], F32, kind="ExternalInput").ap()
    out = nc.dram_tensor("out", [N_IMG, IMG_DIM], F16 if out16 else F32,
                         kind="ExternalOutput").ap()

    with tile.TileContext(nc) as tc:
        with ExitStack() as ctx:
            if tiny:
                with tc.tile_pool(name="tp", bufs=1) as tp:
                    tt = tp.tile([P, 512], F16, tag="tt")
                    nc.sync.dma_start(tt, img[:P, :])
                    nc.sync.dma_start(out[:P, :], tt)
            else:
                _body(ctx, tc, img, txt, msk, wq, wk, wv, wo, bo, out, njc,
                      repeat, xpose16, out16)
    nc.compile()
    return nc


def _body(ctx, tc, img, txt, msk, wq, wk, wv, wo, bo, out, njc, repeat=1,
          xpose16=True, out16=True):
    nc = tc.nc
    Exp = mybir.ActivationFunctionType.Exp

    img_r = img.rearrange("(n p) d -> p n d", p=P)  # n = 32 row-chunks
    out_r = out.rearrange("(n p) d -> p n d", p=P)

    const = ctx.enter_context(tc.tile_pool(name="const", bufs=1))
    ps = ctx.enter_context(tc.tile_pool(name="ps", bufs=8, space="PSUM"))

    identity = const.tile([P, P], F32, tag="identity")
    make_identity(nc, identity)
    identity_h = const.tile([P, P], F16, tag="identity_h")
    make_identity(nc, identity_h)

    # ---- constants / weights (transposed into [contract-dim, free] layouts)
    WqT = const.tile([P, 4, 512], F32R, tag="WqT")  # [d, qd]
    WoT = const.tile([P, 4, 512], F32R, tag="WoT")  # [c, od]
    WkT = const.tile([P, 6, 512], F32R, tag="WkT")  # [td, kd]
    WvT = const.tile([P, 6, 512], F32R, tag="WvT")  # [td, vd]
    tT = const.tile([P, 6, njc * P], F32R, tag="tT")    # [td, j]
    KT = const.tile([P, 4, njc * P], F32R, tag="KT")    # [kd, j]
    Vx = const.tile([P, njc, H, 2 * HD], F32R, tag="Vx")  # [j%, jc, h, vd|mask]
    bo_sb = const.tile([1, 512], F32, tag="bo_sb")
    bo_r = const.tile([1, 512], F32R, tag="bo_r")
    ones = const.tile([1, P], F32R, tag="ones")
    ones_f = const.tile([P, HD], F32, tag="ones_f")
    nc.any.memset(ones_f, 1.0)
    # mask as multiplicative factor on V_ext rows, laid out [p, jc].
    # Load contiguously as [njc, 128] (njc descriptors) and PE-transpose;
    # a direct [p, jc] DMA would be njc*128 4-byte descriptors.
    maskb_f = const.tile([P, njc], F32, tag="mf")
    mask_row = const.tile([njc, P], F32, tag="mrow")
    nc.sync.dma_start(mask_row, msk.rearrange("(c p) -> c p", p=P))
    mps = ps.tile([P, njc], F32, tag="ps", bufs=6, name="mps")
    nc.tensor.transpose(mps, mask_row, identity[:njc, :njc])
    nc.vector.tensor_copy(maskb_f, mps)
    nc.vector.tensor_copy(ones, ones_f[0:1, 0:1].broadcast_to([1, P]))
    for jc in range(njc):
        nc.vector.tensor_scalar_mul(
            Vx[:, jc, :, HD:],
            ones_f[:, None, :].broadcast_to([P, H, HD]),
            maskb_f[:, jc : jc + 1],
        )
    nc.gpsimd.dma_start(bo_sb, bo.unsqueeze(0))
    nc.vector.tensor_copy(bo_r, bo_sb)

    def transpose_in(dst, src_chunks, n_out_chunks, n_in_chunks, evict_engine):
        """dst[p, oc, ic*128+q] = src[q, ic, oc*128+p]; src is fp16,
        transposed through PE into an fp16 view of an f32 PSUM tile, then
        upcast to f32 on eviction."""
        for oc in range(n_out_chunks):
            pst = ps.tile([P, 512], F32, tag="ps", bufs=6, name=f"pst_{oc}")
            pst16 = pst.bitcast(F16)
            for ic in range(n_in_chunks):
                nc.tensor.transpose(
                    pst16[:, ic * P : (ic + 1) * P],
                    src_chunks[:, ic, oc * P : (oc + 1) * P],
                    identity_h,
                )
            evict_engine.tensor_copy(
                dst[:, oc, : n_in_chunks * P], pst16[:, : n_in_chunks * P]
            )

    def transpose_in_f32(dst, src_chunks, n_out_chunks, n_in_chunks, evict_engine):
        for oc in range(n_out_chunks):
            pst = ps.tile([P, 512], F32, tag="ps", bufs=6, name=f"pstf_{oc}")
            for ic in range(n_in_chunks):
                nc.tensor.transpose(
                    pst[:, ic * P : (ic + 1) * P],
                    src_chunks[:, ic, oc * P : (oc + 1) * P],
                    identity,
                )
            evict_engine.tensor_copy(
                dst[:, oc, : n_in_chunks * P], pst[:, : n_in_chunks * P]
            )

    # ---- one-time setup: text/weight transposes, K^T, V
    wload = ctx.enter_context(tc.tile_pool(name="wload", bufs=2))
    t_sb = wload.tile([P, 4, 768], F16, tag="wl")
    nc.sync.dma_start(t_sb[:, :njc, :], txt.rearrange("(c p) d -> p c d", p=P))
    transpose_in(tT, t_sb[:, :njc, :], 6, njc, nc.vector)

    wk_sb = wload.tile([P, 4, 768], F16, tag="wl")
    nc.scalar.dma_start(wk_sb, wk.rearrange("(c p) d -> p c d", p=P))
    transpose_in(WkT, wk_sb, 6, 4, nc.vector)

    wq_sb = wload.tile([P, 4, 768], F16, tag="wl")
    nc.sync.dma_start(wq_sb[:, :, :512], wq.rearrange("(c p) d -> p c d", p=P))
    transpose_in(WqT, wq_sb[:, :, :512], 4, 4, nc.vector)

    wv_sb = wload.tile([P, 4, 768], F16, tag="wl")
    nc.scalar.dma_start(wv_sb, wv.rearrange("(c p) d -> p c d", p=P))
    transpose_in(WvT, wv_sb, 6, 4, nc.vector)

    wo_sb = wload.tile([P, 4, 768], F16, tag="wl")
    nc.sync.dma_start(wo_sb[:, :, :512], wo.rearrange("(c p) d -> p c d", p=P))
    transpose_in(WoT, wo_sb[:, :, :512], 4, 4, nc.vector)

    # K^T[kd, j] = sum_td WkT[td, kd] * tT[td, j]
    for kc in range(4):
        pkt = ps.tile([P, 512], F32, tag="ps", bufs=6, name=f"pkt_{kc}")
        for t6 in range(6):
            nc.tensor.matmul(
                pkt[:, : njc * P],
                WkT[:, t6, kc * P : (kc + 1) * P],
                tT[:, t6, :],
                start=(t6 == 0),
                stop=(t6 == 5),
            )
        nc.vector.tensor_copy(KT[:, kc, :], pkt[:, : njc * P])

    # V[j, vd] = sum_td tT[td, j] * WvT[td, vd]; per-head columns, mask applied
    for jc in range(njc):
        pv = ps.tile([P, 512], F32, tag="ps", bufs=6, name=f"pv_{jc}")
        for t6 in range(6):
            nc.tensor.matmul(
                pv,
                tT[:, t6, jc * P : (jc + 1) * P],
                WvT[:, t6, :],
                start=(t6 == 0),
                stop=(t6 == 5),
            )
        nc.vector.tensor_scalar_mul(
            Vx[:, jc, :, :HD],
            pv.rearrange("p (h v) -> p h v", h=H),
            maskb_f[:, jc : jc + 1],
        )

    # ---- pipelined pools for the main loop
    xload = ctx.enter_context(tc.tile_pool(name="xload", bufs=2))
    xtp = ctx.enter_context(tc.tile_pool(name="xtp", bufs=2))
    qtp = ctx.enter_context(tc.tile_pool(name="qtp", bufs=2))
    exp = ctx.enter_context(tc.tile_pool(name="exw", bufs=3))
    anp = ctx.enter_context(tc.tile_pool(name="anp", bufs=2))
    ysp = ctx.enter_context(tc.tile_pool(name="ysp", bufs=3))
    rcp = ctx.enter_context(tc.tile_pool(name="rcp", bufs=3))

    def _main_loop():
      for ib in range(IB):
        x_sb = xload.tile([P, 4, 512], F16, tag="x")
        nc.sync.dma_start(x_sb, img_r[:, ib * 4 : (ib + 1) * 4, :])

        # x^T for this 512-query block
        xT = xtp.tile([P, 4, 512], F32R, tag="xT")  # [d, i]
        if xpose16:
            transpose_in(xT, x_sb, 4, 4, nc.vector)
        else:
            x32 = xload.tile([P, 4, 512], F32, tag="x32")
            nc.gpsimd.tensor_copy(x32, x_sb)
            transpose_in_f32(xT, x32, 4, 4, nc.vector)

        # Q^T[qd, i] = sum_d WqT[d, qd] * xT[d, i]
        qt = qtp.tile([P, 4, 512], F32R, tag="qt")  # [qd, i]
        for qc in range(4):
            pq = ps.tile([P, 512], F32, tag="ps", bufs=6, name=f"pq_{qc}")
            for dc in range(4):
                nc.tensor.matmul(
                    pq,
                    WqT[:, dc, qc * P : (qc + 1) * P],
                    xT[:, dc, :],
                    start=(dc == 0),
                    stop=(dc == 3),
                )
            nc.vector.tensor_copy(qt[:, qc, :], pq)

        attn = anp.tile([P, 4, 512], F32R, tag="attn")  # [c, i] normalized att^T

        def head_scores(h):
            po = (h % 2) * HD
            hc = h // 2
            qh = qt[po : po + HD, hc, :]  # [64, 512]
            ex = exp.tile([P, njc, 512], F32R, tag="ex", name="ex")
            for jc in range(njc):
                sc = ps.tile([P, 512], F32, tag="ps", bufs=6, name=f"sc_{jc}")
                nc.tensor.matmul(
                    sc,
                    KT[po : po + HD, hc, jc * P : (jc + 1) * P],
                    qh,
                )
                nc.scalar.activation(ex[:, jc, :], sc, Exp, scale=SCALE)
            return ex

        def head_attend(h, ex):
            po = (h % 2) * HD
            hc = h // 2
            at = ps.tile([P, 512], F32, tag="at", bufs=2, name="at")
            for jc in range(njc):
                nc.tensor.matmul(
                    at,
                    Vx[:, jc, h, :],
                    ex[:, jc, :],
                    start=(jc == 0),
                    stop=(jc == njc - 1),
                )
            # rows [HD:2*HD] of `at` are the softmax denominator, replicated
            rec = rcp.tile([HD, 512], F32, tag="rec")
            nc.vector.reciprocal(rec, at[HD:, :])
            nc.vector.tensor_mul(attn[po : po + HD, hc, :], at[:HD, :], rec)

        # software pipeline: head h's scores/exp overlap head h-1's attend
        prev = None
        for h in range(H):
            ex = head_scores(h)
            if prev is not None:
                head_attend(prev[0], prev[1])
            prev = (h, ex)
        head_attend(prev[0], prev[1])

        # Y[i, od] = sum_c attn[c, i] * WoT[c, od] + bo
        for mc in range(4):
            py = ps.tile([P, 512], F32, tag="ps", bufs=6, name=f"py_{mc}")
            for cc in range(4):
                nc.tensor.matmul(
                    py,
                    attn[:, cc, mc * P : (mc + 1) * P],
                    WoT[:, cc, :],
                    start=(cc == 0),
                    stop=False,
                )
            nc.tensor.matmul(py, ones[0:1, :], bo_r, start=False, stop=True)
            y_sb = ysp.tile([P, 512], F16 if out16 else F32, tag="y")
            nc.scalar.copy(y_sb, py)
            nc.scalar.dma_start(out_r[:, ib * 4 + mc, :], y_sb)

    if repeat == 1:
        _main_loop()
    else:
        with tc.For_i(0, repeat, 1):
            _main_loop()


# ---------------------------------------------------------------------------
# host-side runner


_libc = ctypes.CDLL("libc.so.6", use_errno=True)
_libc.memcmp.argtypes = [ctypes.c_void_p, ctypes.c_void_p, ctypes.c_size_t]
_libc.memcmp.restype = ctypes.c_int


def _memeq(a: np.ndarray, b: np.ndarray) -> bool:
    if a.shape != b.shape or a.dtype != b.dtype:
        return False
    return (
        _libc.memcmp(
            ctypes.c_void_p(a.ctypes.data),
            ctypes.c_void_p(b.ctypes.data),
            a.nbytes,
        )
        == 0
    )


_MESH = None


def _mesh():
    global _MESH
    if _MESH is None:
        import jax
        from jax.sharding import Mesh

        _MESH = Mesh(np.asarray(jax.devices()[:N_CORES]), ("core",))
    return _MESH


_RUNNERS = {}


def _get_runner(nj: int = NJ, repeat: int = 1, xpose16: bool = True, out16: bool = True):
    """Build the Bass program once per (nj, repeat) and wrap it in a cached
    8-core shard_map jit. Output buffers are allocated on device inside the
    jitted body (no host zeros upload)."""
    key = (nj, repeat, xpose16, out16)
    if key in _RUNNERS:
        return _RUNNERS[key]

    import jax
    import jax.numpy as jnp
    from jax.sharding import PartitionSpec
    from jax.experimental.shard_map import shard_map
    from concourse import bass2jax

    nc = _build_nc(nj if nj > 0 else NJ, tiny=(nj <= 0), repeat=repeat,
                   xpose16=xpose16, out16=out16)
    bass2jax.install_neuronx_cc_hook()

    partition_name = nc.partition_id_tensor.name if nc.partition_id_tensor else None
    in_names = []
    out_names = []
    out_avals = []
    out_shapes = []
    for alloc in nc.m.functions[0].allocations:
        if not isinstance(alloc, mybir.MemoryLocationSet):
            continue
        name = alloc.memorylocations[0].name
        if alloc.kind == "ExternalInput":
            if name != partition_name:
                in_names.append(name)
        elif alloc.kind == "ExternalOutput":
            shape = tuple(alloc.tensor_shape)
            dtype = mybir.dt.np(alloc.dtype)
            out_names.append(name)
            out_avals.append(jax.core.ShapedArray(shape, dtype))
            out_shapes.append((shape, dtype))
    n_params = len(in_names)
    n_outs = len(out_names)
    all_names = list(in_names) + list(out_names)
    if partition_name is not None:
        all_names.append(partition_name)

    def _bodyfn(*args):
        operands = list(args)
        if partition_name is not None:
            operands.append(bass2jax.partition_id_tensor())
        outs = bass2jax._bass_exec_p.bind(
            *operands,
            out_avals=tuple(out_avals),
            in_names=tuple(all_names),
            out_names=tuple(out_names),
            lowering_input_output_aliases=(),
            sim_require_finite=True,
            sim_require_nnan=True,
            nc=nc,
        )
        return tuple(outs)

    # Donating img lets XLA alias the fp16 output onto img's buffer (same
    # per-core shape/dtype). A non-aliased fresh output buffer makes the
    # NEFF's output DMA writes ~6x slower; img is consumed fresh every call
    # anyway, so this costs nothing. In-place is safe: each 512-row block of
    # img is read strictly before that block's out rows are written, and
    # distinct blocks touch disjoint rows.
    sharded = jax.jit(
        shard_map(
            _bodyfn,
            mesh=_mesh(),
            in_specs=(PartitionSpec("core"),) * (n_params + n_outs),
            out_specs=(PartitionSpec("core"),) * n_outs,
            check_rep=False,
        ),
        keep_unused=True,
        donate_argnums=(in_names.index("img"),),
    )

    _RUNNERS[key] = (sharded, in_names, out_names, out_shapes, nc)
    return _RUNNERS[key]


_ZCACHE = {}


def _zeros_dev(out_shapes):
    """Device-resident, non-donated ballast for the Bass program's output
    operands. The neuron lowering never reads their content (the NEFF binds
    its outputs to the result buffers), so one cached array serves every
    call with zero per-call transfer."""
    key = tuple(out_shapes)
    if key in _ZCACHE:
        return _ZCACHE[key]
    import jax
    import jax.numpy as jnp
    from jax.sharding import NamedSharding, PartitionSpec

    sh = NamedSharding(_mesh(), PartitionSpec("core"))
    zs = []
    for shape, dtype in out_shapes:
        gshape = (N_CORES * shape[0],) + tuple(shape[1:])
        z = jax.jit(lambda g=gshape, d=dtype: jnp.zeros(g, d), out_shardings=sh)()
        zs.append(z)
    jax.block_until_ready(zs)
    _ZCACHE[key] = zs
    return zs


def _compact_text(text_embeds: np.ndarray, msk: np.ndarray):
    """Gather valid text tokens (mask != 0) to the front, pad to NJ, fp16.
    Softmax gives masked tokens zero weight, so dropping them is exact.
    Falls back to the uncompacted 512-key layout if some batch has > NJ
    valid tokens. Returns (nj, txt16 (B*nj,768), val (B*nj,) f32)."""
    t = np.asarray(text_embeds)
    valid = msk != 0
    if valid.sum(axis=1).max() <= NJ:
        nj = NJ
        txt16 = np.zeros((B, NJ, TEXT_DIM), np.float16)
        val = np.zeros((B, NJ), np.float32)
        for b in range(B):
            ix = np.nonzero(valid[b])[0]
            txt16[b, : len(ix)] = t[b][ix]
            val[b, : len(ix)] = 1.0
    else:
        nj = N_TXT
        txt16 = np.asarray(t, dtype=np.float16)
        val = valid.astype(np.float32)
    return nj, txt16.reshape(B * nj, TEXT_DIM), val.reshape(B * nj)


_WCACHE = {}


def _weights_dev(Wq, Wk, Wv, Wo, bo):
    """fp16 weights, replicated per core, cached device-resident. An exact
    memcmp against the previous call's f32 values guards the cache."""
    import jax
    from jax.sharding import NamedSharding, PartitionSpec

    sh = NamedSharding(_mesh(), PartitionSpec("core"))
    out = {}
    for name, w, dt in (
        ("wq", Wq, np.float16),
        ("wk", Wk, np.float16),
        ("wv", Wv, np.float16),
        ("wo", Wo, np.float16),
        ("bo", bo, np.float32),
    ):
        w = np.ascontiguousarray(np.asarray(w, dtype=np.float32))
        ent = _WCACHE.get(name)
        if ent is not None and _memeq(ent[0], w):
            out[name] = ent[1]
            continue
        wc = np.asarray(w, dtype=dt)
        rep = np.broadcast_to(wc, (N_CORES,) + wc.shape).reshape(
            (N_CORES * wc.shape[0],) + wc.shape[1:]
        )
        dev = jax.device_put(rep, sh)
        jax.block_until_ready(dev)
        _WCACHE[name] = (w.copy(), dev)
        out[name] = dev
    return out


def _prep_inputs(img_embeds, text_embeds, text_attention_mask, Wq, Wk, Wv, Wo, bo):
    """Returns (feed dict name->global array, nj)."""
    img = np.ascontiguousarray(np.asarray(img_embeds, dtype=np.float32))
    msk = np.asarray(text_attention_mask)
    nj, txt16, val = _compact_text(text_embeds, msk)
    img16 = img.reshape(B * N_IMG, IMG_DIM).astype(np.float16)
    feed = {"img": img16, "txt": txt16, "msk": val}
    feed.update(_weights_dev(Wq, Wk, Wv, Wo, bo))
    return feed, nj


def kernel(img_embeds, text_embeds, text_attention_mask, Wq, Wk, Wv, Wo, bo):
    feed, nj = _prep_inputs(
        img_embeds, text_embeds, text_attention_mask, Wq, Wk, Wv, Wo, bo
    )
    sharded, in_names, out_names, out_shapes, _ = _get_runner(nj)
    outs = sharded(*(feed[n] for n in in_names), *_zeros_dev(out_shapes))
    out16 = np.asarray(outs[out_names.index("out")])
    return out16.astype(np.float32).reshape(B, N_IMG, IMG_DIM)


def bench_repeat(feed, nj: int = NJ, repeat: int = 25, iters: int = 12,
                 xpose16: bool = True, out16: bool = True):
    """Device-time via an in-NEFF For_i repeat loop: (t[repeat] - t[1]) /
    (repeat - 1). The repeat variant runs the whole main loop `repeat` times
    on device inside one dispatch, so the delta is pure device time."""
    import time
    import jax
    from jax.sharding import NamedSharding, PartitionSpec

    sh = NamedSharding(_mesh(), PartitionSpec("core"))
    runs = {}
    for rep in (1, repeat):
        sharded, in_names, _, out_shapes, _ = _get_runner(nj, rep, xpose16, out16)
        img_pos = in_names.index("img")
        dev_in = [
            a if isinstance(a, jax.Array) else jax.device_put(a, sh)
            for a in (feed[n] for n in in_names)
        ] + list(_zeros_dev(out_shapes))
        jax.block_until_ready(dev_in)
        imgnp = np.asarray(feed[in_names[img_pos]])
        runs[rep] = (sharded, dev_in, img_pos, imgnp)

    def call(rep, timed):
        sharded, dev_in, img_pos, imgnp = runs[rep]
        # img is donated: re-stage a fresh device copy outside the timed
        # region so the timed dispatch measures floor + device time only
        dev_img = jax.device_put(imgnp, sh)
        jax.block_until_ready(dev_img)
        args = list(dev_in)
        args[img_pos] = dev_img
        t0 = time.perf_counter()
        o = sharded(*args)
        jax.block_until_ready(o)
        dt = time.perf_counter() - t0
        if timed:
            times[rep].append(dt)

    times = {1: [], repeat: []}
    for rep in (1, repeat):
        call(rep, False)
    for _ in range(iters):
        for rep in (1, repeat):
            call(rep, True)
    per = (min(times[repeat]) - min(times[1])) / (repeat - 1)
    return per, times
